# revision 26
# baseline (speedup 1.0000x reference)
# Multi-head attention kernel for Trainium2 (Bass/Tile), 8-core SPMD.
#
# Problem: B=4, S=2048, D=1024, H=16 heads, d_k=64 (fp32 in/out).
#
# Sharding: core c = (batch b, head-group g) with b = c//2, g = c%2.
# Each core computes 8 heads of one batch entirely on-device and emits the
# partial final projection (out_heads @ Wo_slice^T) over the full model dim.
# Host sums the two partial outputs per batch (the "all-reduce" of the
# tensor-parallel Wo) and adds the linear bias terms.
#
# All matmul operands are bf16 (fp32 accumulation in PSUM): bf16 weights
# get the PE fast-weight-load path and halve the DMA streams; the measured
# output error stays ~8e-3 relative, well inside the 2e-2 gate.
#
# Dataflow (per core) avoids every on-device transpose:
#   - host feeds x^T [D, S] so the contraction dim (d) is on partitions
#   - Q^T, K^T [e, s] computed directly (e on partitions)
#   - V [s, e] computed naturally (s on partitions), pre-scaled by the key
#     mask, with the mask itself appended as a 65th column per head so the
#     attention-V matmul also produces the softmax denominators (row 64).
#   - scores computed transposed S^T[k, q] = K^T.T-chunks @ Q^T, two heads
#     per kc-group on disjoint 64-row tile_position groups (they execute
#     concurrently in the PE array).
#   - exp on ScalarE straight out of PSUM in [128, 1024] batches; the
#     attention pipeline is software-pipelined (scores run two kc-groups
#     ahead of exp, attn@V trails exp) and the (qc0, hp0) slice of it is
#     interleaved into phase 1 as its K/V s-chunks land.
#   - attn@V via lhsT = [V*mask | mask] (M=65), accumulated over 16 k-chunks
#   - softmax normalization: reciprocal of row 64, gpsimd partition
#     broadcast (library preloaded at t=0), one DVE multiply per head.
#   - final^T[e, q] = Wo^T-chunks @ O^T accumulated over the 4 local
#     d-chunks; each qc's projection is deferred one qc and its column
#     chunks fill the PE slack at the next qc's hp boundaries.
#
# Biases: bq/bk added on device (per-partition adds folded into the PSUM
# eviction). bv and bo are linear post-softmax terms: since softmax rows sum
# to one, (attn@V + bv)@Wo^T + bo == attn@V@Wo^T + (bv@Wo^T + bo), which the
# host adds to the gathered output.

from contextlib import ExitStack

import numpy as np

import concourse.bass as bass  # noqa: F401  (AP types come via handles)
import concourse.tile as tile
from concourse import bacc, library_config, mybir
from concourse.bass_utils import run_bass_kernel_spmd

P = 128
S = 2048          # sequence length
D = 1024          # model dim
E = 512           # per-core head dims (8 heads x 64)
NH = 8            # heads per core
NDCH = D // P     # 8 contraction chunks for projections
NST = S // P      # 16 s-tiles (key chunks)
NSC = 4           # s-chunks of 512
NET = E // P      # 4 e-tiles of the local head dims
NHP = NH // 2     # 4 head pairs
NKC = NST         # 16 key chunks of 128
NQC = 4           # query chunks of 512
QCW = S // NQC    # 512
VW = 65           # V columns per head incl. mask column

F32 = mybir.dt.float32
F32R = mybir.dt.float32r
BF16 = mybir.dt.bfloat16
AF = mybir.ActivationFunctionType

EXP_GRP = 3       # scores tiles per exp instruction (3 PSUM banks)


def _mm(nc, out, lhsT, rhs, start, stop):
    nc.tensor.matmul(
        out,
        lhsT,
        rhs,
        start=start,
        stop=stop,
    )


def _build_program():
    nc = bacc.Bacc(
        "TRN2",
        debug=False,
        target_bir_lowering=False,
        enable_partition_id=False,
    )

    xT = nc.dram_tensor("xT", [D, S], BF16, kind="ExternalInput").ap()
    wqT = nc.dram_tensor("wqT", [D, E], BF16, kind="ExternalInput").ap()
    wkT = nc.dram_tensor("wkT", [D, E], BF16, kind="ExternalInput").ap()
    wvT = nc.dram_tensor("wvT", [D, E], BF16, kind="ExternalInput").ap()
    woT = nc.dram_tensor("woT", [E, D], BF16, kind="ExternalInput").ap()
    bq_t = nc.dram_tensor("bq_t", [P, NET], F32, kind="ExternalInput").ap()
    bk_t = nc.dram_tensor("bk_t", [P, NET], F32, kind="ExternalInput").ap()
    mk_t = nc.dram_tensor("mk_t", [P, NST], F32, kind="ExternalInput").ap()
    mask8 = nc.dram_tensor("mask8", [NST, P, NH], BF16, kind="ExternalInput").ap()
    fT = nc.dram_tensor("fT", [D, S], BF16, kind="ExternalOutput").ap()

    with tile.TileContext(nc) as tc, ExitStack() as ctx:
        pers = ctx.enter_context(tc.tile_pool(name="pers", bufs=1))

        # load the Q7 library for partition_broadcast up front: the implicit
        # load at first use costs ~7us mid-pipeline and cold-restarts the PE.
        nc.gpsimd.load_library(library_config.attn)

        KT = [pers.tile([P, S], BF16, name=f"KT{j}", tag=f"KT{j}") for j in range(NET)]
        QT = [pers.tile([P, S], BF16, name=f"QT{j}", tag=f"QT{j}") for j in range(NET)]
        Vg = [
            pers.tile([P, NH * VW], BF16, name=f"Vg{t}", tag=f"Vg{t}")
            for t in range(NST)
        ]
        bq_sb = pers.tile([P, NET], F32, name="bq_sb", tag="bq_sb")
        bk_sb = pers.tile([P, NET], F32, name="bk_sb", tag="bk_sb")
        mk_sb = pers.tile([P, NST], F32, name="mk_sb", tag="mk_sb")
        nc.gpsimd.dma_start(bq_sb[:], bq_t)
        nc.gpsimd.dma_start(bk_sb[:], bk_t)
        nc.gpsimd.dma_start(mk_sb[:], mk_t)

        # ---------------- Unified pipeline ----------------
        # Phase 1 (projections) and phase 2 (attention) share one pool scope
        # so the (qc0, hp0) attention pipeline can interleave into phase 1 as
        # its KT/QT/Vg s-chunks land: ScalarE starts exp ~75us earlier.
        # PSUM budget: ppsum 2 + spsum 2x2 + vpsum 2 = 8; deferred
        # wo-projection chunks reuse the (idle) ppsum banks in phase 2.
        qwp = ctx.enter_context(tc.tile_pool(name="qwp", bufs=1))
        qw = [qwp.tile([P, E], BF16, name=f"qw{d}", tag=f"qw{d}") for d in range(NDCH)]
        wpool = ctx.enter_context(tc.tile_pool(name="wpool", bufs=1))
        xpool = ctx.enter_context(tc.tile_pool(name="xpool", bufs=12))
        wopool = ctx.enter_context(tc.tile_pool(name="wopool", bufs=1))
        ptpool = ctx.enter_context(tc.tile_pool(name="ptpool", bufs=4))
        otpool = ctx.enter_context(tc.tile_pool(name="otpool", bufs=2))
        npool = ctx.enter_context(tc.tile_pool(name="npool", bufs=2))
        ostage = ctx.enter_context(tc.tile_pool(name="ostage", bufs=3))
        vpsum = ctx.enter_context(tc.tile_pool(name="vpsum", bufs=2, space="PSUM"))
        pctx = ctx.enter_context(ExitStack())
        ppsum = pctx.enter_context(tc.tile_pool(name="ppsum", bufs=2, space="PSUM"))
        spsum = pctx.enter_context(tc.tile_pool(name="spsum", bufs=2, space="PSUM"))

        kw = [wpool.tile([P, E], BF16, name=f"kw{d}", tag=f"kw{d}") for d in range(NDCH)]
        vw = [wpool.tile([P, E], BF16, name=f"vw{d}", tag=f"vw{d}") for d in range(NDCH)]
        wo = [
            wopool.tile([P, D], BF16, name=f"wo{c}", tag=f"wo{c}") for c in range(4)
        ]
        # weights stream on the scalar HWDGE queue (ScalarE is idle in phase
        # 1); x and qw ride sync/gpsimd. wo is emitted after the x stream.
        for d in range(NDCH):
            nc.scalar.dma_start(kw[d][:, 0 : 2 * P], wkT[d * P : (d + 1) * P, 0 : 2 * P])
        for d in range(NDCH):
            nc.scalar.dma_start(kw[d][:, 2 * P :], wkT[d * P : (d + 1) * P, 2 * P :])

        # ---- attention pipeline state (flat kc-group list) ----
        # only the (qc0, hp0) pipeline interleaves into phase 1 (2-wide exp
        # groups from spsum); everything else runs post-phase-1 at 3-wide
        # exp groups once the projection PSUM banks are released.
        groups = [(0, 0, kc) for kc in range(NKC)]
        NG = len(groups)
        st_tiles = {}
        state = {"next": 0, "proc": 0, "pending": None, "pv": None, "acc": {}}
        OTs_of = {}

        def avail(i):
            return groups[i][2] // 4

        def emit_scores(i):
            gqc, ghp, gkc = groups[i]
            gqsl = slice(gqc * QCW, (gqc + 1) * QCW)
            st = spsum.tile([P, QCW * 2], F32, name="st", tag="st")
            for h in (0, 1):
                lo = h * 64
                _mm(
                    nc,
                    st[:, h * QCW : (h + 1) * QCW],
                    KT[ghp][lo : lo + 64, gkc * P : (gkc + 1) * P],
                    QT[ghp][lo : lo + 64, gqsl],
                    start=True,
                    stop=True,
                )
            st_tiles[i] = st

        def pump_scores(i_max, sc_done):
            while (
                state["next"] < NG
                and state["next"] <= i_max
                and avail(state["next"]) <= sc_done
            ):
                emit_scores(state["next"])
                state["next"] += 1

        def emit_wo_chunk(qc_prev, OTs_prev, j, eng=None):
            qsl_p = slice(qc_prev * QCW, (qc_prev + 1) * QCW)
            wops = ppsum.tile([P, QCW], F32, name="pps", tag="pps")
            for hp in range(NHP):
                _mm(
                    nc,
                    wops[:],
                    wo[hp][:, j * P : (j + 1) * P],
                    OTs_prev[hp][:],
                    start=(hp == 0),
                    stop=(hp == NHP - 1),
                )
            ot = ostage.tile([P, QCW], BF16, name="os", tag="os")
            nc.vector.tensor_copy(ot[:], wops[:])
            if eng is None:
                eng = nc.sync if j % 2 == 0 else nc.gpsimd
            eng.dma_start(fT[j * P : (j + 1) * P, qsl_p], ot[:])

        def process_group(i, sc_done):
            qc, hp, kc = groups[i]
            if hp == 0 and kc == 0:
                OTs_of[qc] = [
                    otpool.tile([P, QCW], BF16, name=f"ot{h2}", tag=f"ot{h2}")
                    for h2 in range(NHP)
                ]
            if kc == 0:
                pvA = vpsum.tile([P, QCW], F32, name="pv", tag="pv")
                pvB = vpsum.tile([P, QCW], F32, name="pv", tag="pv")
                state["pv"] = (pvA, pvB)
            pv_of = state["pv"]

            st = st_tiles.pop(i)
            pt = ptpool.tile([P, QCW * 2], BF16, name="pt", tag="pt")
            nc.scalar.activation(pt[:], st[:], AF.Exp, scale=0.125)
            pump_scores(i + 2, sc_done)
            if kc == NKC - 1 and state["pending"] is not None:
                # fill the PE slack at this hp boundary with two deferred
                # output-projection chunks (ScalarE keeps streaming exp).
                qp, OTp = state["pending"]
                emit_wo_chunk(qp, OTp, hp * 2)
                emit_wo_chunk(qp, OTp, hp * 2 + 1)
            for h in (0, 1):
                hh = hp * 2 + h
                _mm(
                    nc,
                    pv_of[h][0:VW, :],
                    Vg[kc][:, hh * VW : (hh + 1) * VW],
                    pt[:, h * QCW : (h + 1) * QCW],
                    start=(kc == 0),
                    stop=(kc == NKC - 1),
                )
            if kc != NKC - 1:
                return

            emit_norm(qc, hp, pv_of)

            if hp == NHP - 1:
                state["pending"] = (qc, OTs_of.pop(qc))

        def emit_norm(qc, hp, pv_of):
            # softmax normalization; head A -> OT rows 0-63, head B -> OT
            # rows 64-127 (via SBUF->SBUF DMA). PV psum banks are evicted to
            # SBUF immediately so they recycle fast; the rest of the chain
            # runs off the critical path. HW quirks: partition_broadcast
            # reads the source tile's physical partition 0 and writes from
            # partition 0 only, so the reciprocal row is shifted to
            # partition 0 first (single-input DVE copies may shift base).
            pvA, pvB = pv_of
            pvsA = npool.tile([P, QCW], F32, name="pvsA", tag="pvsA")
            pvsB = npool.tile([P, QCW], F32, name="pvsB", tag="pvsB")
            nc.vector.tensor_copy(pvsA[0:VW, :], pvA[0:VW, :])
            nc.vector.tensor_copy(pvsB[0:VW, :], pvB[0:VW, :])
            rpA = npool.tile([P, QCW], F32, name="rpA", tag="rpA", bufs=1)
            rpB = npool.tile([P, QCW], F32, name="rpB", tag="rpB", bufs=1)
            rcA = npool.tile([P, QCW], F32, name="rcA", tag="rcA", bufs=1)
            rcB = npool.tile([P, QCW], F32, name="rcB", tag="rcB", bufs=1)
            bcA = npool.tile([P, QCW], F32, name="bcA", tag="bcA", bufs=1)
            bcB = npool.tile([P, QCW], F32, name="bcB", tag="bcB", bufs=1)
            tmB = npool.tile([P, QCW], BF16, name="tmB", tag="tmB")
            # custom DVE ops misbehave off base partition 0 on HW: shift the
            # sums row down first, then approx-recip at partition 0.
            nc.vector.tensor_copy(rpA[0:1, :], pvsA[64:65, :])
            nc.vector.tensor_copy(rpB[0:1, :], pvsB[64:65, :])
            nc.vector.reciprocal_approx_fast(rcA[0:1, :], rpA[0:1, :])
            nc.vector.reciprocal_approx_fast(rcB[0:1, :], rpB[0:1, :])
            nc.gpsimd.partition_broadcast(bcA[0:64, :], rcA[0:1, :], channels=64)
            nc.gpsimd.partition_broadcast(bcB[0:64, :], rcB[0:1, :], channels=64)
            nc.vector.tensor_mul(OTs_of[qc][hp][0:64, :], pvsA[0:64, :], bcA[0:64, :])
            nc.vector.tensor_mul(tmB[0:64, :], pvsB[0:64, :], bcB[0:64, :])
            nc.sync.dma_start(OTs_of[qc][hp][64:128, :], tmB[0:64, :])

        def advance(sc_done):
            while state["proc"] < NG and avail(state["proc"]) <= sc_done:
                i = state["proc"]
                pump_scores(i + 1, sc_done)
                process_group(i, sc_done)
                state["proc"] += 1

        # ---- phase 1: projections, with the qc0/hp0 pipeline riding along
        for sc in range(NSC):
            ssl = slice(sc * QCW, (sc + 1) * QCW)
            xs = []
            for d in range(NDCH):
                xt = xpool.tile([P, QCW], BF16, name="xt", tag="xt")
                eng = nc.sync if d % 2 == 0 else nc.gpsimd
                eng.dma_start(xt[:], xT[d * P : (d + 1) * P, ssl])
                xs.append(xt)
            if sc == 0:
                for d in range(NDCH):
                    eng = nc.sync if d % 2 == 0 else nc.gpsimd
                    eng.dma_start(qw[d][:], wqT[d * P : (d + 1) * P, :])
                for d in range(NDCH):
                    nc.scalar.dma_start(vw[d][:], wvT[d * P : (d + 1) * P, :])

            # K^T / Q^T e-tiles: out[e(128), s(512)] = W^T-chunk.T @ x^T
            for W, bias_sb, OUT in ((kw, bk_sb, KT), (qw, bq_sb, QT)):
                for j in range(NET):
                    ps = ppsum.tile([P, QCW], F32, name="pps", tag="pps")
                    for d in range(NDCH):
                        _mm(
                            nc,
                            ps[:],
                            W[d][:, j * P : (j + 1) * P],
                            xs[d][:],
                            start=(d == 0),
                            stop=(d == NDCH - 1),
                        )
                    nc.vector.tensor_scalar_add(
                        OUT[j][:, ssl], ps[:], bias_sb[:, j : j + 1]
                    )

            # V s-tiles: out[s(128), e(512)] = x^T-chunk.T @ Wv^T-chunk
            for t4 in range(4):
                t = sc * 4 + t4
                ps = ppsum.tile([P, QCW], F32, name="pps", tag="pps")
                for d in range(NDCH):
                    _mm(
                        nc,
                        ps[:],
                        xs[d][:, t4 * P : (t4 + 1) * P],
                        vw[d][:],
                        start=(d == 0),
                        stop=(d == NDCH - 1),
                    )
                vdst = Vg[t][:].rearrange("p (h c) -> p h c", c=VW)
                nc.vector.tensor_scalar_mul(
                    vdst[:, :, 0:64],
                    ps[:].rearrange("p (h c) -> p h c", c=64),
                    mk_sb[:, t : t + 1],
                )
                nc.gpsimd.dma_start(vdst[:, :, 64], mask8[t])

            if sc < NSC - 1:
                advance(sc)

        # wo after the x stream on the (now idle) sync queue
        for c in range(4):
            nc.sync.dma_start(wo[c][:], woT[c * P : (c + 1) * P, :])

        # ---- finish the merged (qc0, hp0) slice, release the projection
        # and 2-wide score banks, then run everything else with 3-wide exp
        # groups (fewer ScalarE instruction overheads) and wo chunks on the
        # freed attention banks.
        advance(NSC - 1)
        pctx.close()
        spsum3 = ctx.enter_context(
            tc.tile_pool(name="spsum3", bufs=2, space="PSUM")
        )

        pgroups = []
        for qc in range(NQC):
            for hp in range(NHP):
                if qc == 0 and hp == 0:
                    continue
                us = [(kc, h) for kc in range(NKC) for h in (0, 1)]
                for k in range(0, len(us), 3):
                    pgroups.append((qc, hp, us[k : k + 3]))
        st3 = {}

        def emit_scores3(gi):
            gqc, ghp, gus = pgroups[gi]
            gqsl = slice(gqc * QCW, (gqc + 1) * QCW)
            st = spsum3.tile([P, QCW * 3], F32, name="st3", tag="st3")
            for ii, (kc, h) in enumerate(gus):
                lo = h * 64
                _mm(
                    nc,
                    st[:, ii * QCW : (ii + 1) * QCW],
                    KT[ghp][lo : lo + 64, kc * P : (kc + 1) * P],
                    QT[ghp][lo : lo + 64, gqsl],
                    start=True,
                    stop=True,
                )
            st3[gi] = st

        def emit_wo_chunk3(qp, OTp, j, eng=None):
            wops = vpsum.tile([P, QCW], F32, name="pv", tag="pv")
            for hp2 in range(NHP):
                _mm(
                    nc,
                    wops[:],
                    wo[hp2][:, j * P : (j + 1) * P],
                    OTp[hp2][:],
                    start=(hp2 == 0),
                    stop=(hp2 == NHP - 1),
                )
            ot = ostage.tile([P, QCW], BF16, name="os", tag="os")
            nc.vector.tensor_copy(ot[:], wops[:])
            if eng is None:
                eng = nc.sync if j % 2 == 0 else nc.gpsimd
            eng.dma_start(fT[j * P : (j + 1) * P, qp * QCW : (qp + 1) * QCW], ot[:])

        emit_scores3(0)
        emit_scores3(1)
        for gi, (qc, hp, us) in enumerate(pgroups):
            if us[0] == (0, 0):
                if hp == 0:
                    OTs_of[qc] = [
                        otpool.tile([P, QCW], BF16, name=f"ot{h2}", tag=f"ot{h2}")
                        for h2 in range(NHP)
                    ]
                pvA = vpsum.tile([P, QCW], F32, name="pv", tag="pv")
                pvB = vpsum.tile([P, QCW], F32, name="pv", tag="pv")
                state["pv"] = (pvA, pvB)
            pv_of = state["pv"]
            st = st3.pop(gi)
            nw = len(us) * QCW
            pt = ptpool.tile([P, QCW * 3], BF16, name="pt3", tag="pt3", bufs=4)
            nc.scalar.activation(pt[:, :nw], st[:, :nw], AF.Exp, scale=0.125)
            if gi + 2 < len(pgroups):
                emit_scores3(gi + 2)
            for ii, (kc, h) in enumerate(us):
                hh = hp * 2 + h
                _mm(
                    nc,
                    pv_of[h][0:VW, :],
                    Vg[kc][:, hh * VW : (hh + 1) * VW],
                    pt[:, ii * QCW : (ii + 1) * QCW],
                    start=(kc == 0),
                    stop=(kc == NKC - 1),
                )
            if us[-1] != (NKC - 1, 1):
                continue
            emit_norm(qc, hp, pv_of)
            if state["pending"] is not None:
                qp, OTp = state["pending"]
                emit_wo_chunk3(qp, OTp, hp * 2)
                emit_wo_chunk3(qp, OTp, hp * 2 + 1)
            if hp == NHP - 1:
                state["pending"] = (qc, OTs_of.pop(qc))

        # flush the last qc's projection
        qp, OTp = state["pending"]
        flush_engs = (nc.sync, nc.gpsimd, nc.scalar)
        for j in range(D // P):
            emit_wo_chunk3(qp, OTp, j, eng=flush_engs[j % 3])

    nc.compile()
    return nc


_PROGRAM = None


def _get_program():
    global _PROGRAM
    if _PROGRAM is None:
        _PROGRAM = _build_program()
    return _PROGRAM


def make_in_maps(x, mask, Wq, Wk, Wv, bq, bk):
    """Per-core input dicts. Core c: batch c//2, head-group c%2."""
    import ml_dtypes

    bf16 = ml_dtypes.bfloat16
    WqT = np.ascontiguousarray(Wq.T.astype(bf16))
    WkT = np.ascontiguousarray(Wk.T.astype(bf16))
    WvT = np.ascontiguousarray(Wv.T.astype(bf16))
    in_maps = []
    for c in range(8):
        b, g = divmod(c, 2)
        esl = slice(g * E, (g + 1) * E)
        m = mask[b].astype(np.float32)
        mk = np.ascontiguousarray(m.reshape(NST, P).T)
        m8 = np.ascontiguousarray(
            np.repeat(m.reshape(NST, P, 1), NH, axis=2).astype(bf16)
        )
        in_maps.append(
            {
                "xT": np.ascontiguousarray(x[b].T.astype(bf16)),
                "wqT": np.ascontiguousarray(WqT[:, esl]),
                "wkT": np.ascontiguousarray(WkT[:, esl]),
                "wvT": np.ascontiguousarray(WvT[:, esl]),
                "bq_t": np.ascontiguousarray(bq[esl].reshape(NET, P).T.astype(np.float32)),
                "bk_t": np.ascontiguousarray(bk[esl].reshape(NET, P).T.astype(np.float32)),
                "mk_t": mk,
                "mask8": m8,
            }
        )
    return in_maps


def kernel(**inputs):
    x = np.asarray(inputs["x"], dtype=np.float32)
    mask = np.asarray(inputs["mask"])
    Wq = np.asarray(inputs["Wq"], dtype=np.float32)
    Wk = np.asarray(inputs["Wk"], dtype=np.float32)
    Wv = np.asarray(inputs["Wv"], dtype=np.float32)
    Wo = np.asarray(inputs["Wo"], dtype=np.float32)
    bq = np.asarray(inputs["bq"], dtype=np.float32)
    bk = np.asarray(inputs["bk"], dtype=np.float32)
    bv = np.asarray(inputs["bv"], dtype=np.float32)
    bo = np.asarray(inputs["bo"], dtype=np.float32)

    nc = _get_program()

    import ml_dtypes

    WoT = np.ascontiguousarray(Wo.T)  # [d, e]
    WoT16 = WoT.astype(ml_dtypes.bfloat16)
    in_maps = make_in_maps(x, mask, Wq, Wk, Wv, bq, bk)
    for c in range(8):
        g = c % 2
        in_maps[c]["woT"] = np.ascontiguousarray(WoT16[g * E : (g + 1) * E, :])

    res = run_bass_kernel_spmd(nc, in_maps, core_ids=list(range(8)))

    extra = (bv @ WoT + bo).astype(np.float32)  # [D]
    out = np.empty((4, S, D), dtype=np.float32)
    for b in range(4):
        acc = res.results[2 * b]["fT"].astype(np.float32) + res.results[
            2 * b + 1
        ]["fT"].astype(np.float32)  # [D, S]
        out[b] = acc.T + extra[None, :]
    return out



# revision 28
# speedup vs baseline: 1.0082x; 1.0082x over previous
# Multi-head attention kernel for Trainium2 (Bass/Tile), 8-core SPMD.
#
# Problem: B=4, S=2048, D=1024, H=16 heads, d_k=64 (fp32 in/out).
#
# Sharding: core c = (batch b, head-group g) with b = c//2, g = c%2.
# Each core computes 8 heads of one batch entirely on-device and emits the
# partial final projection (out_heads @ Wo_slice^T) over the full model dim.
# Host sums the two partial outputs per batch (the "all-reduce" of the
# tensor-parallel Wo) and adds the linear bias terms.
#
# All matmul operands are bf16 (fp32 accumulation in PSUM): bf16 weights
# get the PE fast-weight-load path and halve the DMA streams; the measured
# output error stays ~8e-3 relative, well inside the 2e-2 gate.
#
# Dataflow (per core) avoids every on-device transpose:
#   - host feeds x^T [D, S] so the contraction dim (d) is on partitions
#   - Q^T, K^T [e, s] computed directly (e on partitions)
#   - V [s, e] computed naturally (s on partitions), pre-scaled by the key
#     mask, with the mask itself appended as a 65th column per head so the
#     attention-V matmul also produces the softmax denominators (row 64).
#   - scores computed transposed S^T[k, q] = K^T.T-chunks @ Q^T, two heads
#     per kc-group on disjoint 64-row tile_position groups (they execute
#     concurrently in the PE array).
#   - exp on ScalarE straight out of PSUM in [128, 1024] batches; the
#     attention pipeline is software-pipelined (scores run two kc-groups
#     ahead of exp, attn@V trails exp) and the (qc0, hp0) slice of it is
#     interleaved into phase 1 as its K/V s-chunks land.
#   - attn@V via lhsT = [V*mask | mask] (M=65), accumulated over 16 k-chunks
#   - softmax normalization: reciprocal of row 64, gpsimd partition
#     broadcast (library preloaded at t=0), one DVE multiply per head.
#   - final^T[e, q] = Wo^T-chunks @ O^T accumulated over the 4 local
#     d-chunks; each qc's projection is deferred one qc and its column
#     chunks fill the PE slack at the next qc's hp boundaries.
#
# Biases: bq/bk added on device (per-partition adds folded into the PSUM
# eviction). bv and bo are linear post-softmax terms: since softmax rows sum
# to one, (attn@V + bv)@Wo^T + bo == attn@V@Wo^T + (bv@Wo^T + bo), which the
# host adds to the gathered output.

from contextlib import ExitStack

import numpy as np

import concourse.bass as bass  # noqa: F401  (AP types come via handles)
import concourse.tile as tile
from concourse import bacc, library_config, mybir
from concourse.bass_utils import run_bass_kernel_spmd

P = 128
S = 2048          # sequence length
D = 1024          # model dim
E = 512           # per-core head dims (8 heads x 64)
NH = 8            # heads per core
NDCH = D // P     # 8 contraction chunks for projections
NST = S // P      # 16 s-tiles (key chunks)
NSC = 4           # s-chunks of 512
NET = E // P      # 4 e-tiles of the local head dims
NHP = NH // 2     # 4 head pairs
NKC = NST         # 16 key chunks of 128
NQC = 4           # query chunks of 512
QCW = S // NQC    # 512
VW = 65           # V columns per head incl. mask column

F32 = mybir.dt.float32
F32R = mybir.dt.float32r
BF16 = mybir.dt.bfloat16
AF = mybir.ActivationFunctionType

EXP_GRP = 3       # scores tiles per exp instruction (3 PSUM banks)


def _mm(nc, out, lhsT, rhs, start, stop):
    nc.tensor.matmul(
        out,
        lhsT,
        rhs,
        start=start,
        stop=stop,
    )


def _build_program():
    nc = bacc.Bacc(
        "TRN2",
        debug=False,
        target_bir_lowering=False,
        enable_partition_id=False,
    )

    xT = nc.dram_tensor("xT", [D, S], BF16, kind="ExternalInput").ap()
    wqT = nc.dram_tensor("wqT", [D, E], BF16, kind="ExternalInput").ap()
    wkT = nc.dram_tensor("wkT", [D, E], BF16, kind="ExternalInput").ap()
    wvT = nc.dram_tensor("wvT", [D, E], BF16, kind="ExternalInput").ap()
    woT = nc.dram_tensor("woT", [E, D], BF16, kind="ExternalInput").ap()
    bq_t = nc.dram_tensor("bq_t", [P, NET], F32, kind="ExternalInput").ap()
    bk_t = nc.dram_tensor("bk_t", [P, NET], F32, kind="ExternalInput").ap()
    mk_t = nc.dram_tensor("mk_t", [P, NST], F32, kind="ExternalInput").ap()
    mask8 = nc.dram_tensor("mask8", [NST, P, NH], BF16, kind="ExternalInput").ap()
    fT = nc.dram_tensor("fT", [D, S], BF16, kind="ExternalOutput").ap()

    with tile.TileContext(nc) as tc, ExitStack() as ctx:
        pers = ctx.enter_context(tc.tile_pool(name="pers", bufs=1))

        # load the Q7 library for partition_broadcast up front: the implicit
        # load at first use costs ~7us mid-pipeline and cold-restarts the PE.
        nc.gpsimd.load_library(library_config.attn)

        KT = [pers.tile([P, S], BF16, name=f"KT{j}", tag=f"KT{j}") for j in range(NET)]
        QT = [pers.tile([P, S], BF16, name=f"QT{j}", tag=f"QT{j}") for j in range(NET)]
        Vg = [
            pers.tile([P, NH * VW], BF16, name=f"Vg{t}", tag=f"Vg{t}")
            for t in range(NST)
        ]
        bq_sb = pers.tile([P, NET], F32, name="bq_sb", tag="bq_sb")
        bk_sb = pers.tile([P, NET], F32, name="bk_sb", tag="bk_sb")
        mk_sb = pers.tile([P, NST], F32, name="mk_sb", tag="mk_sb")
        nc.gpsimd.dma_start(bq_sb[:], bq_t)
        nc.gpsimd.dma_start(bk_sb[:], bk_t)
        nc.gpsimd.dma_start(mk_sb[:], mk_t)

        # ---------------- Unified pipeline ----------------
        # Phase 1 (projections) and phase 2 (attention) share one pool scope
        # so the (qc0, hp0) attention pipeline can interleave into phase 1 as
        # its KT/QT/Vg s-chunks land: ScalarE starts exp ~75us earlier.
        # PSUM budget: ppsum 2 + spsum 2x2 + vpsum 2 = 8; deferred
        # wo-projection chunks reuse the (idle) ppsum banks in phase 2.
        qwp = ctx.enter_context(tc.tile_pool(name="qwp", bufs=1))
        qw = [qwp.tile([P, E], BF16, name=f"qw{d}", tag=f"qw{d}") for d in range(NDCH)]
        wpool = ctx.enter_context(tc.tile_pool(name="wpool", bufs=1))
        xpool = ctx.enter_context(tc.tile_pool(name="xpool", bufs=12))
        wopool = ctx.enter_context(tc.tile_pool(name="wopool", bufs=1))
        ptpool = ctx.enter_context(tc.tile_pool(name="ptpool", bufs=4))
        otpool = ctx.enter_context(tc.tile_pool(name="otpool", bufs=2))
        npool = ctx.enter_context(tc.tile_pool(name="npool", bufs=2))
        ostage = ctx.enter_context(tc.tile_pool(name="ostage", bufs=3))
        ppsum = ctx.enter_context(tc.tile_pool(name="ppsum", bufs=2, space="PSUM"))
        spsum = ctx.enter_context(tc.tile_pool(name="spsum", bufs=2, space="PSUM"))
        vpsum = ctx.enter_context(tc.tile_pool(name="vpsum", bufs=2, space="PSUM"))

        kw = [wpool.tile([P, E], BF16, name=f"kw{d}", tag=f"kw{d}") for d in range(NDCH)]
        vw = [wpool.tile([P, E], BF16, name=f"vw{d}", tag=f"vw{d}") for d in range(NDCH)]
        wo = [
            wopool.tile([P, D], BF16, name=f"wo{c}", tag=f"wo{c}") for c in range(4)
        ]
        # weights stream on the scalar HWDGE queue (ScalarE is idle in phase
        # 1); x and qw ride sync/gpsimd. wo is emitted after the x stream.
        for d in range(NDCH):
            nc.scalar.dma_start(kw[d][:, 0 : 2 * P], wkT[d * P : (d + 1) * P, 0 : 2 * P])
        for d in range(NDCH):
            nc.scalar.dma_start(kw[d][:, 2 * P :], wkT[d * P : (d + 1) * P, 2 * P :])

        # ---- attention pipeline state (flat kc-group list) ----
        groups = [
            (qc, hp, kc)
            for qc in range(NQC)
            for hp in range(NHP)
            for kc in range(NKC)
        ]
        NG = len(groups)
        st_tiles = {}
        state = {
            "next": 0,
            "proc": 0,
            "pending": None,
            "pv": None,
            "acc": {},
            "qdef": [],
        }
        OTs_of = {}

        def avail(i):
            gqc, ghp, gkc = groups[i]
            if gqc == 0 and ghp == 0:
                return gkc // 4
            return NSC - 1

        def emit_scores(i):
            gqc, ghp, gkc = groups[i]
            gqsl = slice(gqc * QCW, (gqc + 1) * QCW)
            st = spsum.tile([P, QCW * 2], F32, name="st", tag="st")
            for h in (0, 1):
                lo = h * 64
                _mm(
                    nc,
                    st[:, h * QCW : (h + 1) * QCW],
                    KT[ghp][lo : lo + 64, gkc * P : (gkc + 1) * P],
                    QT[ghp][lo : lo + 64, gqsl],
                    start=True,
                    stop=True,
                )
            st_tiles[i] = st

        def pump_scores(i_max, sc_done):
            while (
                state["next"] < NG
                and state["next"] <= i_max
                and avail(state["next"]) <= sc_done
            ):
                emit_scores(state["next"])
                state["next"] += 1

        def emit_wo_chunk(qc_prev, OTs_prev, j, eng=None):
            qsl_p = slice(qc_prev * QCW, (qc_prev + 1) * QCW)
            wops = ppsum.tile([P, QCW], F32, name="pps", tag="pps")
            for hp in range(NHP):
                _mm(
                    nc,
                    wops[:],
                    wo[hp][:, j * P : (j + 1) * P],
                    OTs_prev[hp][:],
                    start=(hp == 0),
                    stop=(hp == NHP - 1),
                )
            ot = ostage.tile([P, QCW], BF16, name="os", tag="os")
            nc.vector.tensor_copy(ot[:], wops[:])
            if eng is None:
                eng = nc.sync if j % 2 == 0 else nc.gpsimd
            eng.dma_start(fT[j * P : (j + 1) * P, qsl_p], ot[:])

        def process_group(i, sc_done):
            qc, hp, kc = groups[i]
            if hp == 0 and kc == 0:
                OTs_of[qc] = [
                    otpool.tile([P, QCW], BF16, name=f"ot{h2}", tag=f"ot{h2}")
                    for h2 in range(NHP)
                ]
            if kc == 0:
                pvA = vpsum.tile([P, QCW], F32, name="pv", tag="pv")
                pvB = vpsum.tile([P, QCW], F32, name="pv", tag="pv")
                state["pv"] = (pvA, pvB)
            pv_of = state["pv"]

            st = st_tiles.pop(i)
            pt = ptpool.tile([P, QCW * 2], BF16, name="pt", tag="pt")
            nc.scalar.activation(pt[:], st[:], AF.Exp, scale=0.125)
            pump_scores(i + 2, sc_done)
            if state["qdef"] and (qc, hp) != (0, 0):
                state["qdef"].pop(0)()
            if kc == NKC - 1 and state["pending"] is not None:
                # fill the PE slack at this hp boundary with two deferred
                # output-projection chunks (ScalarE keeps streaming exp).
                qp, OTp = state["pending"]
                emit_wo_chunk(qp, OTp, hp * 2)
                emit_wo_chunk(qp, OTp, hp * 2 + 1)
            for h in (0, 1):
                hh = hp * 2 + h
                _mm(
                    nc,
                    pv_of[h][0:VW, :],
                    Vg[kc][:, hh * VW : (hh + 1) * VW],
                    pt[:, h * QCW : (h + 1) * QCW],
                    start=(kc == 0),
                    stop=(kc == NKC - 1),
                )
            if kc != NKC - 1:
                return

            # softmax normalization; head A -> OT rows 0-63, head B -> OT
            # rows 64-127 (via SBUF->SBUF DMA). PV psum banks are evicted to
            # SBUF immediately so they recycle fast; the rest of the chain
            # runs off the critical path. HW quirks: partition_broadcast
            # reads the source tile's physical partition 0 and writes from
            # partition 0 only, so the reciprocal row is shifted to
            # partition 0 first (single-input DVE copies may shift base).
            pvA, pvB = pv_of
            pvsA = npool.tile([P, QCW], F32, name="pvsA", tag="pvsA")
            pvsB = npool.tile([P, QCW], F32, name="pvsB", tag="pvsB")
            nc.vector.tensor_copy(pvsA[0:VW, :], pvA[0:VW, :])
            nc.vector.tensor_copy(pvsB[0:VW, :], pvB[0:VW, :])
            rpA = npool.tile([P, QCW], F32, name="rpA", tag="rpA", bufs=1)
            rpB = npool.tile([P, QCW], F32, name="rpB", tag="rpB", bufs=1)
            rcA = npool.tile([P, QCW], F32, name="rcA", tag="rcA", bufs=1)
            rcB = npool.tile([P, QCW], F32, name="rcB", tag="rcB", bufs=1)
            bcA = npool.tile([P, QCW], F32, name="bcA", tag="bcA", bufs=1)
            bcB = npool.tile([P, QCW], F32, name="bcB", tag="bcB", bufs=1)
            tmB = npool.tile([P, QCW], BF16, name="tmB", tag="tmB")
            # custom DVE ops misbehave off base partition 0 on HW: shift the
            # sums row down first, then approx-recip at partition 0.
            nc.vector.tensor_copy(rpA[0:1, :], pvsA[64:65, :])
            nc.vector.tensor_copy(rpB[0:1, :], pvsB[64:65, :])
            nc.vector.reciprocal_approx_fast(rcA[0:1, :], rpA[0:1, :])
            nc.vector.reciprocal_approx_fast(rcB[0:1, :], rpB[0:1, :])
            nc.gpsimd.partition_broadcast(bcA[0:64, :], rcA[0:1, :], channels=64)
            nc.gpsimd.partition_broadcast(bcB[0:64, :], rcB[0:1, :], channels=64)
            nc.vector.tensor_mul(OTs_of[qc][hp][0:64, :], pvsA[0:64, :], bcA[0:64, :])
            nc.vector.tensor_mul(tmB[0:64, :], pvsB[0:64, :], bcB[0:64, :])
            nc.sync.dma_start(OTs_of[qc][hp][64:128, :], tmB[0:64, :])

            if hp == NHP - 1:
                state["pending"] = (qc, OTs_of.pop(qc))

        def advance(sc_done):
            while state["proc"] < NG and avail(state["proc"]) <= sc_done:
                i = state["proc"]
                pump_scores(i + 1, sc_done)
                process_group(i, sc_done)
                state["proc"] += 1

        # ---- phase 1: projections, with the qc0/hp0 pipeline riding along
        for sc in range(NSC):
            ssl = slice(sc * QCW, (sc + 1) * QCW)
            xs = []
            for d in range(NDCH):
                xt = xpool.tile([P, QCW], BF16, name="xt", tag="xt")
                eng = nc.sync if d % 2 == 0 else nc.gpsimd
                eng.dma_start(xt[:], xT[d * P : (d + 1) * P, ssl])
                xs.append(xt)
            if sc == 0:
                for d in range(NDCH):
                    eng = nc.sync if d % 2 == 0 else nc.gpsimd
                    eng.dma_start(qw[d][:], wqT[d * P : (d + 1) * P, :])
                for d in range(NDCH):
                    nc.scalar.dma_start(vw[d][:], wvT[d * P : (d + 1) * P, :])

            # K^T / Q^T e-tiles: out[e(128), s(512)] = W^T-chunk.T @ x^T
            # Q(sc3) is only read by qc3's scores much later, so its chains
            # are deferred into the first post-merge attention groups.
            def q_chain(j, xs=xs, ssl=ssl):
                ps = ppsum.tile([P, QCW], F32, name="pps", tag="pps")
                for d in range(NDCH):
                    _mm(
                        nc,
                        ps[:],
                        qw[d][:, j * P : (j + 1) * P],
                        xs[d][:],
                        start=(d == 0),
                        stop=(d == NDCH - 1),
                    )
                nc.vector.tensor_scalar_add(
                    QT[j][:, ssl], ps[:], bq_sb[:, j : j + 1]
                )

            if sc == NSC - 1:
                proj_sets = ((kw, bk_sb, KT),)
                state["qdef"] = [
                    (lambda j=j: q_chain(j)) for j in range(NET)
                ]
            else:
                proj_sets = ((kw, bk_sb, KT), (qw, bq_sb, QT))
            for W, bias_sb, OUT in proj_sets:
                for j in range(NET):
                    ps = ppsum.tile([P, QCW], F32, name="pps", tag="pps")
                    for d in range(NDCH):
                        _mm(
                            nc,
                            ps[:],
                            W[d][:, j * P : (j + 1) * P],
                            xs[d][:],
                            start=(d == 0),
                            stop=(d == NDCH - 1),
                        )
                    nc.vector.tensor_scalar_add(
                        OUT[j][:, ssl], ps[:], bias_sb[:, j : j + 1]
                    )

            # V s-tiles: out[s(128), e(512)] = x^T-chunk.T @ Wv^T-chunk
            for t4 in range(4):
                t = sc * 4 + t4
                ps = ppsum.tile([P, QCW], F32, name="pps", tag="pps")
                for d in range(NDCH):
                    _mm(
                        nc,
                        ps[:],
                        xs[d][:, t4 * P : (t4 + 1) * P],
                        vw[d][:],
                        start=(d == 0),
                        stop=(d == NDCH - 1),
                    )
                vdst = Vg[t][:].rearrange("p (h c) -> p h c", c=VW)
                nc.vector.tensor_scalar_mul(
                    vdst[:, :, 0:64],
                    ps[:].rearrange("p (h c) -> p h c", c=64),
                    mk_sb[:, t : t + 1],
                )
                nc.gpsimd.dma_start(vdst[:, :, 64], mask8[t])

            if sc < NSC - 1:
                advance(sc)

        # wo after the x stream on the (now idle) sync queue
        for c in range(4):
            nc.sync.dma_start(wo[c][:], woT[c * P : (c + 1) * P, :])

        # ---- phase 2: drain the rest of the attention pipeline
        advance(NSC - 1)

        # flush the last qc's projection (ScalarE is idle here, so its DMA
        # queue helps drain the final output chunks)
        qp, OTp = state["pending"]
        flush_engs = (nc.sync, nc.gpsimd, nc.scalar)
        for j in range(D // P):
            emit_wo_chunk(qp, OTp, j, eng=flush_engs[j % 3])

    nc.compile()
    return nc


_PROGRAM = None


def _get_program():
    global _PROGRAM
    if _PROGRAM is None:
        _PROGRAM = _build_program()
    return _PROGRAM


def make_in_maps(x, mask, Wq, Wk, Wv, bq, bk):
    """Per-core input dicts. Core c: batch c//2, head-group c%2."""
    import ml_dtypes

    bf16 = ml_dtypes.bfloat16
    WqT = np.ascontiguousarray(Wq.T.astype(bf16))
    WkT = np.ascontiguousarray(Wk.T.astype(bf16))
    WvT = np.ascontiguousarray(Wv.T.astype(bf16))
    in_maps = []
    for c in range(8):
        b, g = divmod(c, 2)
        esl = slice(g * E, (g + 1) * E)
        m = mask[b].astype(np.float32)
        mk = np.ascontiguousarray(m.reshape(NST, P).T)
        m8 = np.ascontiguousarray(
            np.repeat(m.reshape(NST, P, 1), NH, axis=2).astype(bf16)
        )
        in_maps.append(
            {
                "xT": np.ascontiguousarray(x[b].T.astype(bf16)),
                "wqT": np.ascontiguousarray(WqT[:, esl]),
                "wkT": np.ascontiguousarray(WkT[:, esl]),
                "wvT": np.ascontiguousarray(WvT[:, esl]),
                "bq_t": np.ascontiguousarray(bq[esl].reshape(NET, P).T.astype(np.float32)),
                "bk_t": np.ascontiguousarray(bk[esl].reshape(NET, P).T.astype(np.float32)),
                "mk_t": mk,
                "mask8": m8,
            }
        )
    return in_maps


def kernel(**inputs):
    x = np.asarray(inputs["x"], dtype=np.float32)
    mask = np.asarray(inputs["mask"])
    Wq = np.asarray(inputs["Wq"], dtype=np.float32)
    Wk = np.asarray(inputs["Wk"], dtype=np.float32)
    Wv = np.asarray(inputs["Wv"], dtype=np.float32)
    Wo = np.asarray(inputs["Wo"], dtype=np.float32)
    bq = np.asarray(inputs["bq"], dtype=np.float32)
    bk = np.asarray(inputs["bk"], dtype=np.float32)
    bv = np.asarray(inputs["bv"], dtype=np.float32)
    bo = np.asarray(inputs["bo"], dtype=np.float32)

    nc = _get_program()

    import ml_dtypes

    WoT = np.ascontiguousarray(Wo.T)  # [d, e]
    WoT16 = WoT.astype(ml_dtypes.bfloat16)
    in_maps = make_in_maps(x, mask, Wq, Wk, Wv, bq, bk)
    for c in range(8):
        g = c % 2
        in_maps[c]["woT"] = np.ascontiguousarray(WoT16[g * E : (g + 1) * E, :])

    res = run_bass_kernel_spmd(nc, in_maps, core_ids=list(range(8)))

    extra = (bv @ WoT + bo).astype(np.float32)  # [D]
    out = np.empty((4, S, D), dtype=np.float32)
    for b in range(4):
        acc = res.results[2 * b]["fT"].astype(np.float32) + res.results[
            2 * b + 1
        ]["fT"].astype(np.float32)  # [D, S]
        out[b] = acc.T + extra[None, :]
    return out

